# revision 1
# baseline (speedup 1.0000x reference)
"""GQA causal attention (S=2048, H=32, KVH=8, D=128) on 8 TRN2 NeuronCores.

Sharding: tensor-parallel over heads. Core i computes query heads
[4i, 4i+4) against KV head i (GQA group size 32/8 = 4). No collectives:
the host slices the inputs per core and concatenates the outputs.

Per-core algorithm (seq=2048, d=128, 4 q-heads, 1 kv-head, causal):
  - K^T and per-head Q^T staged in SBUF as [d=128, seq] bf16 via PE
    transposes (fp32 DMA-transpose unsupported; xbar transpose too slow).
  - Scores S^T are packed TIGHTLY across key-tiles into a contiguous
    per-head column space of 17408 = sum_kt (2048-128*kt) columns: the
    causal span of key-tile kt starts right where the previous span
    ends. Score production walks this space in 1024-col chunks through
    two [128,1024] PSUM tiles (2 banks each); QK matmuls split at PSUM
    bank boundaries (one matmul may not cross a bank).
  - exp() runs on ScalarE as ONE wide ACTIVATE per 1024-col chunk,
    ignoring key-tile boundaries, writing a contiguous per-head P^T
    "arena" [128, 17408] bf16 in SBUF (double-buffered across heads).
    68 wide EXPs instead of 96 narrower ones cut the ~200ns/instr
    ScalarE overhead; ScalarE runs near back-to-back (~72us, the
    second-busiest engine after the PE's ~80us).
  - Diagonal 128-col blocks are causal-masked by a 0/1 upper-tri DVE
    multiply as soon as their 128 columns are exp'd (one chunk earlier
    than full-span completion), so PV never waits on a fresh mask.
  - PV: for each query-tile qt: acc[qt] = sum_k2 (arena slice).T @ [V|1]
    in a per-qt PSUM accumulator from a 2-buf pool (banks alternate, so
    the PE's next accumulation never shares a bank with the DVE's
    normalization reads -- PE-W + DVE-R on one bank serializes); the
    DVE normalizes straight out of PSUM (reciprocal of the ones-column
    denominator + tensor_scalar_mul into the store buffer).
    Accumulation groups are atomic: no other matmul may interleave, or
    the PSUM accumulate bits are corrupted.
  - PV lags score production by 3 key-tiles (1 on the last head, which
    with per-qt output stores shortens the kernel tail) and flows
    across head boundaries; the next head's Q prep is spread over late
    chunks; dummy warmup matmuls hold the PE clock at 2.4 GHz through
    the DMA prologue.
"""

import numpy as np

SEQ = 2048
D = 128
QH = 4  # query heads per core
N_CORES = 8
SCALE = 0.08838834764831845  # 1/sqrt(128)
NT = SEQ // 128  # 16 tiles of 128 along seq

RING_SLOTS = 5          # (unused in v3 pool scheme; kept for reference)
SLOT = 512              # fp32 per PSUM bank
CHUNK = 1024            # score chunk: one pool tile (2 banks), one EXP
ARENA = sum(SEQ - 128 * t for t in range(NT))  # 17408 packed score cols/head
NCH = ARENA // CHUNK    # 17 chunks per head

_NC = None


def _emit(ctx, tc, q, k, v, out):
    import concourse.mybir as mybir
    from concourse import masks

    nc = tc.nc
    f32 = mybir.dt.float32
    bf16 = mybir.dt.bfloat16
    Exp = mybir.ActivationFunctionType.Exp

    spans = [SEQ - 128 * t for t in range(NT)]
    offs = [0] * NT
    for t in range(1, NT):
        offs[t] = offs[t - 1] + spans[t - 1]

    singles = ctx.enter_context(tc.tile_pool(name="singles", bufs=1))
    qpool = ctx.enter_context(tc.tile_pool(name="qpool", bufs=2))
    apool = ctx.enter_context(tc.tile_pool(name="apool", bufs=2))
    opool = ctx.enter_context(tc.tile_pool(name="opool", bufs=3))
    # PSUM budget (8 banks): scores 2x2 + PV accumulator 2x1 + transposes 2x1.
    psum_s = ctx.enter_context(tc.tile_pool(name="psum_s", bufs=2, space="PSUM"))
    psum_o = ctx.enter_context(tc.tile_pool(name="psum_o", bufs=2, space="PSUM"))
    psum_t = ctx.enter_context(tc.tile_pool(name="psum_t", bufs=2, space="PSUM"))

    # ---- PE warmup: dense dummy matmuls while the DMA prep runs, so the
    # HAM clock-gate reaches 2.4 GHz by the time real PE work arrives.
    warm_src = singles.tile([128, 512], bf16, tag="warm_src")
    nc.vector.memset(warm_src[:], 0.0)
    for i in range(6):
        wsp = psum_s.tile([128, CHUNK], f32, tag="s")
        for j in range(2):
            nc.tensor.matmul(
                wsp[:, j * 512:(j + 1) * 512], lhsT=warm_src[:, 0:128],
                rhs=warm_src[:], start=True, stop=True,
            )

    ident = singles.tile([128, 128], bf16)
    masks.make_identity(nc, ident[:])
    keep = singles.tile([128, 128], bf16)
    masks.make_upper_triangular(nc, keep[:], val=1.0, diag=True)

    kT = singles.tile([128, SEQ], bf16)
    knat = singles.tile([128, NT, 128], f32, tag="knat")
    knat_bf = singles.tile([128, NT, 128], bf16, tag="knat_bf")
    kr = k.rearrange("(t p) d -> p t d", p=128)

    def kchunk(c):
        """Load + cast + PE-transpose one 4-tile chunk of K into kT.
        The 4 transposes batch into one PSUM bank + one wide DVE copy."""
        cs = slice(c * 4, (c + 1) * 4)
        nc.sync.dma_start(out=knat[:, cs, :], in_=kr[:, cs, :])
        nc.vector.tensor_copy(knat_bf[:, cs, :], knat[:, cs, :])
        for t in range(c * 4, (c + 1) * 4):
            pst = psum_t.tile([128, 128], bf16, tag="tp")
            nc.tensor.transpose(pst[:], knat_bf[:, t, :], ident[:])
            nc.vector.tensor_copy(kT[:, t * 128:(t + 1) * 128], pst[:])

    def qprep_alloc(h):
        qnat = singles.tile([128, NT, 128], f32, tag=f"qnat{h}")
        qnat_bf = singles.tile([128, NT, 128], bf16, tag=f"qnat_bf{h}")
        qT = qpool.tile([128, SEQ], bf16, tag="qT")
        return qnat, qnat_bf, qT

    def qprep_chunk(h, st, c):
        """Load + cast + PE-transpose one 4-tile chunk of head h's Q."""
        qnat, qnat_bf, qT = st
        qrh = q[:, h * D:(h + 1) * D].rearrange("(t p) d -> p t d", p=128)
        cs = slice(c * 4, (c + 1) * 4)
        nc.sync.dma_start(out=qnat[:, cs, :], in_=qrh[:, cs, :])
        nc.vector.tensor_copy(qnat_bf[:, cs, :], qnat[:, cs, :])
        for t in range(c * 4, (c + 1) * 4):
            pst = psum_t.tile([128, 128], bf16, tag="tp")
            nc.tensor.transpose(pst[:], qnat_bf[:, t, :], ident[:])
            nc.vector.tensor_copy(qT[:, t * 128:(t + 1) * 128], pst[:])

    # ---- V: natural [128, t, d] bf16 + ones column for the denominator
    vp = singles.tile([128, NT, D + 1], bf16)
    vnat = singles.tile([128, NT, 128], f32, tag="vnat")

    def vprep():
        nc.sync.dma_start(out=vnat[:], in_=v.rearrange("(t p) d -> p t d", p=128))
        nc.vector.tensor_copy(vp[:, :, 0:D], vnat[:])
        nc.vector.memset(vp[:, :, D:D + 1], 1.0)

    # Pending-PV work: a queue of (h, qt, arena) plus a cursor that can sit
    # mid-accumulation. Each score chunk emits a budgeted number of PV
    # matmuls so the PE load is smooth and the next head's QK chunks are
    # never stuck behind a burst of large PV groups (which would stall the
    # ScalarE exp pipeline at head transitions).
    pvq = []
    pv_state = {}

    def pv_finish(h2, qt2, ops, osb):
        rec = opool.tile([128, 1], f32, tag="rec")
        nc.vector.reciprocal(rec[:], ops[:, D:D + 1])
        nc.vector.tensor_scalar_mul(osb[:, qt2 % 2, :], ops[:, 0:D], rec[:])
        if h2 == QH - 1 and qt2 >= 14:
            # overlap the last two stores with the trailing normalizations
            nc.sync.dma_start(
                out=out[qt2 * 128:(qt2 + 1) * 128, h2 * D:(h2 + 1) * D],
                in_=osb[:, qt2 % 2, :],
            )
        elif qt2 % 2 == 1:
            qb = qt2 // 2
            nc.sync.dma_start(
                out=out[qb * 256:(qb + 1) * 256, h2 * D:(h2 + 1) * D].rearrange(
                    "(j p) d -> p j d", p=128
                ),
                in_=osb[:],
            )

    def pv_advance(budget_mms, max_groups=2):
        """Pop whole qt groups (accumulation groups must not interleave with
        other matmuls) until the MM budget is spent or the queue empties.
        Capped at max_groups so adjacent small groups never ping-pong the
        two PV accumulator banks against the DVE normalization reads.
        O[qt] = sum_k2 arena[k2-slice].T @ [V|1]."""
        left = budget_mms
        groups = 0
        while left > 0 and groups < max_groups and pvq:
            groups += 1
            h2, qt2, arena2 = pvq.pop(0)
            st = pv_state.setdefault(h2, {})
            ops = psum_o.tile([128, D + 1], f32, tag="o")
            if qt2 % 2 == 0:
                osb = opool.tile([128, 2, D], f32, tag="osb")
                st["osb"] = osb
            for kk in range(qt2 + 1):
                a0 = offs[kk] + (qt2 - kk) * 128
                nc.tensor.matmul(
                    ops[:], lhsT=arena2[:, a0:a0 + 128], rhs=vp[:, kk, :],
                    start=(kk == 0), stop=(kk == qt2),
                )
            left -= qt2 + 1
            pv_finish(h2, qt2, ops, st["osb"])

    # Prologue: K chunk 0 and head-0 Q chunks 0-1 give the shortest path to
    # the first QK matmul; the rest is interleaved into head 0's chunk loop.
    kchunk(0)
    q0st = qprep_alloc(0)
    qprep_chunk(0, q0st, 0)
    qprep_chunk(0, q0st, 1)
    vprep()
    qT = q0st[2]

    for h in range(QH):
        arena = apool.tile([128, ARENA], bf16, tag="arena")
        qT_next = None
        qst_next = None
        done_kt = 0       # key-tiles fully exp'd so far
        done_mask = 0     # key-tiles whose diagonal block is masked
        for ci in range(NCH):
            c0, c1 = ci * CHUNK, (ci + 1) * CHUNK
            # late prologue interleave (head 0 only): K DMAs go early so the
            # casts/transposes never stall the PE mid-head.
            if h == 0:
                if ci == 1:
                    qprep_chunk(0, q0st, 2)
                    qprep_chunk(0, q0st, 3)
                elif ci == 2:
                    kchunk(1)
                elif ci == 4:
                    kchunk(2)
                elif ci == 6:
                    kchunk(3)
            sp = psum_s.tile([128, CHUNK], f32, tag="s")
            # QK matmul fragments: split at PSUM bank boundaries
            qk_mms = 0
            for kt in range(NT):
                s0, s1 = max(c0, offs[kt]), min(c1, offs[kt] + spans[kt])
                if s0 >= s1:
                    continue
                p = s0
                while p < s1:
                    w = min(s1 - p, SLOT - (p % SLOT))
                    qs = kt * 128 + (p - offs[kt])
                    nc.tensor.matmul(
                        sp[:, p - c0:p - c0 + w],
                        lhsT=kT[:, kt * 128:(kt + 1) * 128],
                        rhs=qT[:, qs:qs + w],
                        start=True, stop=True,
                    )
                    p += w
                    qk_mms += 1
            # one wide exp over the whole chunk
            nc.scalar.activation(arena[:, c0:c1], sp[:], Exp, scale=SCALE)
            # mask diagonal blocks as soon as their 128 cols are exp'd (an
            # earlier chunk than full-span completion, so PV never waits on
            # a freshly issued mask), and queue PV on full completion.
            while done_mask < NT and offs[done_mask] + 128 <= c1:
                o0 = offs[done_mask]
                nc.vector.tensor_mul(arena[:, o0:o0 + 128],
                                     arena[:, o0:o0 + 128], keep[:])
                done_mask += 1
            while done_kt < NT and offs[done_kt] + spans[done_kt] <= c1:
                pvq.append((h, done_kt, arena))
                done_kt += 1
            # lag-based PV drain
            lag = 1 if h == QH - 1 else 3
            if len(pvq) > lag:
                pv_advance(17, max_groups=1)
            if h == QH - 1 and len(pvq) > lag:
                pv_advance(17, max_groups=1)
            # spread the next head's Q prep over late chunks
            if h + 1 < QH:
                if ci == 12:
                    qst_next = qprep_alloc(h + 1)
                    qT_next = qst_next[2]
                if 12 <= ci <= 15:
                    qprep_chunk(h + 1, qst_next, ci - 12)
        if qT_next is not None:
            qT = qT_next
    while pvq:
        pv_advance(16, max_groups=1)


def _build():
    import concourse.mybir as mybir
    import concourse.tile as tile
    from concourse import bacc
    from contextlib import ExitStack

    nc = bacc.Bacc()
    q = nc.declare_dram_parameter("q", [SEQ, QH * D], mybir.dt.float32, isOutput=False)
    k = nc.declare_dram_parameter("k", [SEQ, D], mybir.dt.float32, isOutput=False)
    v = nc.declare_dram_parameter("v", [SEQ, D], mybir.dt.float32, isOutput=False)
    out = nc.declare_dram_parameter("out", [SEQ, QH * D], mybir.dt.float32, isOutput=True)

    with tile.TileContext(nc) as tc:
        with ExitStack() as ctx:
            _emit(ctx, tc, q[:], k[:], v[:], out[:])
    nc.compile()
    return nc


def _get_nc():
    global _NC
    if _NC is None:
        _NC = _build()
    return _NC


def _ensure_ntff_hook():
    """The agent image's antenv lacks axon_hooks; shim it so trace=True works."""
    import sys
    import types

    if "antenv.axon_hooks" in sys.modules:
        return
    try:
        import antenv
        from trn_agent_boot.trn_boot import _ntff_profile_via_ctypes
    except ImportError:
        return
    mod = types.ModuleType("antenv.axon_hooks")
    hook = [None]
    mod.set_axon_ntff_profile_hook = lambda h: hook.__setitem__(0, h)
    mod.get_axon_ntff_profile_hook = lambda: hook[0]
    sys.modules["antenv.axon_hooks"] = mod
    antenv.axon_hooks = mod
    mod.set_axon_ntff_profile_hook(_ntff_profile_via_ctypes("/opt/axon/libaxon_pjrt.so"))


def _run(q, k, v, trace=False):
    from concourse.bass_utils import run_bass_kernel_spmd

    if trace:
        _ensure_ntff_hook()
    nc = _get_nc()
    in_maps = []
    for i in range(N_CORES):
        in_maps.append(
            {
                "q": np.ascontiguousarray(q[:, i * QH * D:(i + 1) * QH * D]).astype(np.float32, copy=False),
                "k": np.ascontiguousarray(k[:, i * D:(i + 1) * D]).astype(np.float32, copy=False),
                "v": np.ascontiguousarray(v[:, i * D:(i + 1) * D]).astype(np.float32, copy=False),
            }
        )
    res = run_bass_kernel_spmd(nc, in_maps, core_ids=list(range(N_CORES)), trace=trace)
    full = np.concatenate([res.results[i]["out"] for i in range(N_CORES)], axis=1)
    return full.astype(np.float32, copy=False), res


def kernel(q, k, v):
    out, _ = _run(q, k, v, trace=False)
    return out

